# revision 15
# baseline (speedup 1.0000x reference)
import sys, os
import numpy as np

for p in ("/opt/trn_rl_repo",):
    if p not in sys.path:
        sys.path.insert(0, p)

NC_CAP, DC, ROUT, EPS = 16, 32, 3, 1e-7
B, S, DIN, O = 256, 512, 256, 512   # full problem;  O = NC_CAP*DC
NCORES = 8
BPC = B // NCORES                   # 32 batches per core
NB = 8                              # batches per routing group (free axis = 16*NB)
NG = BPC // NB                      # 4 groups per core

LAST_RESULTS = None


def _split_bir_waits(bir_json):
    """Reduce every instruction to at most one sync-wait command.

    The walrus build in this container rejects instructions with more than
    one sync wait. Two rewrites, semantics preserving:
    1. Drop waits on the instruction's own engine semaphore when another
       wait remains — engines retire instructions in order, so a wait on
       an earlier same-engine instruction is implied by program order.
    2. Split any remaining extra waits onto preceding same-engine Drain
       carriers (AND of conditions == sequential waits).
    """
    import json
    d = json.loads(bir_json)
    engs = ("PE", "DVE", "Activation", "Pool", "SP")
    for f in d["functions"]:
        for bb in f["blocks"]:
            out = []
            for ins in bb["instructions"]:
                si = ins.get("sync_info")
                if si:
                    waits = si.get("on_wait") or []
                    if len(waits) > 1:
                        eng = ins["engine"]
                        own = [w for w in waits
                               if w.get("ant_name", "").startswith(eng + "_")
                               and eng in engs]
                        foreign = [w for w in waits if w not in own]
                        if foreign:
                            waits = foreign
                        else:
                            waits = [waits[-1]]
                    if len(waits) > 1:
                        for i, w in enumerate(waits[:-1]):
                            out.append({
                                "debug": ins.get("debug", 0),
                                "engine": ins["engine"],
                                "ins": [], "outs": [],
                                "name": f"{ins['name']}_w{i}",
                                "opcode": "Drain",
                                "sync_info": {"on_update": [], "on_wait": [w]},
                            })
                        waits = [waits[-1]]
                    si["on_wait"] = waits
                out.append(ins)
            bb["instructions"] = out
    return json.dumps(d).encode()


_COMPILE_PATCHED = False


def _install_compile_patch():
    global _COMPILE_PATCHED
    if _COMPILE_PATCHED:
        return
    import concourse.bass_utils as _bu
    import concourse.bass2jax as _b2j
    _orig = _bu.compile_bir_kernel

    def _patched(bir_json, tmpdir, neff_name="file.neff", **kw):
        return _orig(_split_bir_waits(bir_json), tmpdir, neff_name=neff_name, **kw)

    _bu.compile_bir_kernel = _patched
    _b2j.compile_bir_kernel = _patched
    _COMPILE_PATCHED = True


def _kernel_numpy(u_vecs, W):
    u = u_vecs.astype(np.float32)
    w = W[0].astype(np.float32)
    uh = np.einsum('bsi,io->bso', u, w)
    uh = uh.reshape(B, S, NC_CAP, DC).transpose(0, 2, 1, 3)
    b = np.zeros((B, NC_CAP, S), dtype=np.float32)
    out = None
    for i in range(ROUT):
        m = b.max(axis=1, keepdims=True)
        e = np.exp(b - m)
        c = e / e.sum(axis=1, keepdims=True)
        o = np.einsum('bni,bnid->bnd', c, uh)
        out = o / np.sqrt((o * o).sum(-1, keepdims=True) + EPS)
        if i < ROUT - 1:
            b = np.einsum('bnd,bnid->bni', out, uh)
    return out.astype(np.float32)


def _const_blocks():
    """Host-built constant blocks DMA'd in as extra inputs."""
    import ml_dtypes
    bf16 = ml_dtypes.bfloat16
    # CB (bf16) [128, 289]: ident(128) | c0(128) | psel(32) | ones_col(1)
    cb = np.zeros((128, 289), dtype=np.float32)
    cb[:, 0:128] = np.eye(128, dtype=np.float32)
    cb[:, 128:256] = 1.0 / 16.0
    r = np.arange(128)
    cb[:, 256:288] = (r[:, None] % 32 == np.arange(32)[None, :]).astype(np.float32)
    cb[:, 288] = 1.0
    # CF (f32) [128, 640]: masks 4x128 | ones block 128
    cf = np.zeros((128, 640), dtype=np.float32)
    n_of_col = np.arange(128) % 16
    for ot in range(4):
        m = ((4 * ot + r[:, None] // 32) == n_of_col[None, :]).astype(np.float32)
        cf[:, 128 * ot:128 * (ot + 1)] = m
    cf[:, 512:640] = 1.0
    return cb.astype(bf16), cf


def _build_bass(bpc=BPC):
    import concourse.bass as bass
    import concourse.tile as tile
    from concourse import mybir
    from contextlib import ExitStack

    f32, bf16 = mybir.dt.float32, mybir.dt.bfloat16
    AF = mybir.ActivationFunctionType
    ng = bpc // NB

    nc = bass.Bass()
    u_d = nc.declare_dram_parameter("u", [bpc, S, DIN], f32, isOutput=False)
    w_d = nc.declare_dram_parameter("W", [1, DIN, O], f32, isOutput=False)
    cb_d = nc.declare_dram_parameter("CB", [128, 289], bf16, isOutput=False)
    cf_d = nc.declare_dram_parameter("CF", [128, 640], f32, isOutput=False)
    out_d = nc.declare_dram_parameter("out", [bpc, NC_CAP, DC], f32, isOutput=True)

    with ExitStack() as ctx:
        tc = ctx.enter_context(tile.TileContext(nc))
        const = ctx.enter_context(tc.tile_pool(name="const", bufs=1))
        sb_u = ctx.enter_context(tc.tile_pool(name="sb_u", bufs=1))
        sb_t = ctx.enter_context(tc.tile_pool(name="sb_t", bufs=1))
        sb_c = ctx.enter_context(tc.tile_pool(name="sb_c", bufs=1))
        work = ctx.enter_context(tc.tile_pool(name="work", bufs=2))
        ps_tp = ctx.enter_context(tc.tile_pool(name="ps_tp", bufs=1, space="PSUM"))
        ps_z = ctx.enter_context(tc.tile_pool(name="ps_z", bufs=3, space="PSUM"))
        ps_q = ctx.enter_context(tc.tile_pool(name="ps_q", bufs=4, space="PSUM"))

        # ---- constants ----
        cb = const.tile([128, 289], bf16, tag="cb")
        nc.sync.dma_start(cb[:], cb_d[:])
        cf = const.tile([128, 640], f32, tag="cf")
        nc.sync.dma_start(cf[:], cf_d[:])
        ident = cb[:, 0:128]
        c0 = cb[:, 128:256]
        psel = cb[:, 256:288]
        ones_col = cb[:, 288:289]
        mask3 = cf[:, 0:512].rearrange("p (ot c) -> p ot c", ot=4)
        onesf_row = cf[0:1, 512:640]
        epsc = const.tile([1, 1], f32, tag="epsc")
        nc.vector.memset(epsc[:], EPS)

        # ---- W load, cast, transpose ----
        wf = const.tile([128, 2, 512], f32, tag="wf")
        nc.sync.dma_start(wf[:], w_d[0].rearrange("(it p) o -> p it o", p=128))
        wbt = const.tile([128, 2, 512], bf16, tag="wbt")
        nc.vector.tensor_copy(wbt[:], wf[:])
        wtb = []
        for ot in range(4):
            ptw = ps_tp.tile([128, 2, 512], bf16, tag="tp")
            for it in range(2):
                nc.tensor.transpose(
                    ptw[:, 0, 128 * it:128 * (it + 1)],
                    wbt[:, it, 128 * ot:128 * (ot + 1)], ident)
            t = const.tile([128, 256], bf16, tag=f"wtb{ot}")
            nc.scalar.copy(t[:], ptw[:, 0, 0:256])
            wtb.append(t)

        ub = [None] * bpc
        uts = [None] * bpc
        cT = {}   # g -> [128, 4(st), NB, 16] bf16 tile (c for next iter)

        def prologue(g):
            for l in range(NB):
                b = NB * g + l
                uf = work.tile([128, 4, 256], f32, tag="uf", bufs=3)
                nc.sync.dma_start(uf[:], u_d[b].rearrange("(st p) i -> p st i", p=128))
                t = sb_u.tile([128, 4, 256], bf16, tag=f"ub{b}")
                if b % 3 == 0:
                    nc.vector.tensor_copy(t[:], uf[:])
                else:
                    nc.scalar.copy(t[:], uf[:])
                ub[b] = t
            for l in range(NB):
                b = NB * g + l
                ptp = ps_tp.tile([128, 2, 512], bf16, tag="tp")
                for it in range(2):
                    for st in range(4):
                        nc.tensor.transpose(
                            ptp[:, it, 128 * st:128 * (st + 1)],
                            ub[b][:, st, 128 * it:128 * (it + 1)], ident)
                t = sb_t.tile([128, 2, 512], bf16, tag=f"ut{b}")
                nc.vector.tensor_copy(t[:], ptp[:])
                uts[b] = t

        def crhs(g, k, st, l):
            if k == 0:
                return c0[:, 16 * l:16 * (l + 1)]
            return cT[g][:, st, l, :]

        def routing(g, k):
            # one PSUM bank per group-iter for ZT/norm/bcast/G (regions reused
            # sequentially; Tile subtile deps order the writers/readers)
            zfat = ps_z.tile([128, 3, 128], f32, tag="z")
            pz = zfat[:, 0:2, :]
            # ---- ZT[i,(l,n)] = sum_s u[s,i]*c[s,(l,n)] ----
            for it in range(2):
                for l in range(NB):
                    for st in range(4):
                        nc.tensor.matmul(
                            pz[:, it, 16 * l:16 * (l + 1)],
                            ub[NB * g + l][:, st, 128 * it:128 * (it + 1)],
                            crhs(g, k, st, l),
                            start=(st == 0), stop=(st == 3))
            zb = work.tile([128, 2, 128], bf16, tag="zb", bufs=3)
            nc.vector.tensor_copy(zb[:], pz[:])
            # ---- P[o,(l,n)] = sum_i W[i,o]*ZT[i,(l,n)];  V = P*mask ----
            pp = ps_q.tile([128, 4, 128], f32, tag="q")
            for ot in range(4):
                for it in range(2):
                    nc.tensor.matmul(pp[:, ot, :],
                                     wbt[:, it, 128 * ot:128 * (ot + 1)],
                                     zb[:, it, :], start=(it == 0), stop=(it == 1))
            vb = work.tile([128, 4, 128], bf16, tag="vb", bufs=3)
            nc.vector.tensor_mul(vb[:], pp[:], mask3)
            vsq = work.tile([128, 4, 128], bf16, tag="vsq")
            nc.scalar.activation(vsq[:], vb[:], AF.Square)
            # ---- |V|^2 col sums (accumulate the 4 o-tiles into [1,128]) ----
            pn = zfat[0:1, 2, :]
            for ot in range(4):
                nc.tensor.matmul(pn, ones_col, vsq[:, ot, :],
                                 start=(ot == 0), stop=(ot == 3))
            sq = work.tile([1, 128], f32, tag="sq")
            nc.scalar.activation(sq[:], pn, AF.Sqrt, bias=epsc[:])
            rsn = work.tile([1, 128], f32, tag="rsn")
            nc.vector.reciprocal(rsn[:], sq[:])
            pbc = zfat[:, 2, :]
            nc.tensor.matmul(pbc, onesf_row, rsn[:], start=True, stop=True)
            snsb = work.tile([128, 128], f32, tag="snsb")
            nc.scalar.copy(snsb[:], pbc)
            if k < ROUT - 1:
                # ---- G[i,(l,n)] = sum_o W[i,o]*V[o,(l,n)], scaled by rsqrt ----
                pg = zfat[:, 0:2, :]
                for it in range(2):
                    for ot in range(4):
                        nc.tensor.matmul(pg[:, it, :],
                                         wtb[ot][:, 128 * it:128 * (it + 1)],
                                         vb[:, ot, :], start=(ot == 0), stop=(ot == 3))
                gb = work.tile([128, 2, 128], bf16, tag="gb")
                nc.vector.tensor_mul(
                    gb[:], pg[:], snsb[:].unsqueeze(1).broadcast_to((128, 2, 128)))
                # ---- bT[s,(l,n)] = sum_i u[s,i]*G[i,(l,n)] (= b*rsqrt) ----
                pbt = ps_q.tile([128, 4, 128], f32, tag="q")
                for st in range(4):
                    for l in range(NB):
                        for it in range(2):
                            nc.tensor.matmul(
                                pbt[:, st, 16 * l:16 * (l + 1)],
                                uts[NB * g + l][:, it, 128 * st:128 * (st + 1)],
                                gb[:, it, 16 * l:16 * (l + 1)],
                                start=(it == 0), stop=(it == 1))
                # softmax over n (16-blocks), scale-free (c ~ 16*softmax)
                e = work.tile([128, 4, NB, 16], f32, tag="e", bufs=3)
                nc.scalar.activation(
                    e[:], pbt[:].rearrange("p st (l n) -> p st l n", n=16), AF.Exp)
                d8 = work.tile([128, 4, NB, 8], f32, tag="d8")
                nc.vector.tensor_add(d8[:], e[:, :, :, 0:8], e[:, :, :, 8:16])
                d4 = work.tile([128, 4, NB, 4], f32, tag="d4")
                nc.vector.tensor_add(d4[:], d8[:, :, :, 0:4], d8[:, :, :, 4:8])
                d2 = work.tile([128, 4, NB, 2], f32, tag="d2")
                nc.vector.tensor_add(d2[:], d4[:, :, :, 0:2], d4[:, :, :, 2:4])
                d1 = work.tile([128, 4, NB, 1], f32, tag="d1")
                nc.vector.tensor_add(d1[:], d2[:, :, :, 0:1], d2[:, :, :, 1:2])
                rr = work.tile([128, 4, NB, 1], f32, tag="rr")
                nc.vector.reciprocal(rr[:], d1[:])
                ct = sb_c.tile([128, 4, NB, 16], bf16, tag=f"c{g}_{k % 2}")
                nc.vector.tensor_mul(
                    ct[:], e[:], rr[:].broadcast_to((128, 4, NB, 16)))
                cT[g] = ct
            else:
                # ---- final: vs = V*rsqrt; extract [(l,n), d] via psel ----
                vs = work.tile([128, 4, 128], bf16, tag="vs")
                nc.vector.tensor_mul(
                    vs[:], vb[:], snsb[:].unsqueeze(1).broadcast_to((128, 4, 128)))
                pout = ps_q.tile([128, 4, 128], f32, tag="q")
                for ot in range(4):
                    nc.tensor.matmul(pout[:, 0, 0:32], vs[:, ot, :], psel,
                                     start=(ot == 0), stop=(ot == 3))
                osb = work.tile([128, 32], f32, tag="osb")
                nc.scalar.copy(osb[:], pout[:, 0, 0:32])
                nc.sync.dma_start(
                    out_d[NB * g:NB * (g + 1)].rearrange("b n d -> (b n) d"),
                    osb[:])

        # software-pipelined emission: prologue(g) at step g, iter k at step g+1+k
        for step in range(ng + ROUT):
            for g in range(ng):
                p = step - g
                if p == 0:
                    prologue(g)
                elif 1 <= p <= ROUT:
                    routing(g, p - 1)
    return nc


def kernel(u_vecs, W):
    global LAST_RESULTS
    try:
        _install_compile_patch()
        from concourse.bass_utils import run_bass_kernel_spmd
        nc = _build_bass()
        cb, cf = _const_blocks()
        u = np.ascontiguousarray(u_vecs, dtype=np.float32)
        w = np.ascontiguousarray(W, dtype=np.float32)
        in_maps = [
            {"u": u[c * BPC:(c + 1) * BPC], "W": w, "CB": cb, "CF": cf}
            for c in range(NCORES)
        ]
        res = run_bass_kernel_spmd(nc, in_maps, core_ids=list(range(NCORES)))
        LAST_RESULTS = res
        out = np.concatenate([res.results[c]["out"] for c in range(NCORES)], axis=0)
        return out.astype(np.float32)
    except Exception as ex:
        import traceback
        traceback.print_exc(file=sys.stderr)
        sys.stderr.write(f"[kernel.py] bass path failed ({ex!r}); numpy fallback\n")
        return _kernel_numpy(u_vecs, W)


# revision 17
# speedup vs baseline: 38.6692x; 38.6692x over previous
import sys, os
import numpy as np

for p in ("/opt/trn_rl_repo",):
    if p not in sys.path:
        sys.path.insert(0, p)

NC_CAP, DC, ROUT, EPS = 16, 32, 3, 1e-7
B, S, DIN, O = 256, 512, 256, 512   # full problem;  O = NC_CAP*DC
NCORES = 8
BPC = B // NCORES                   # 32 batches per core
NB = 8                              # batches per routing group (free axis = 16*NB)
NG = BPC // NB                      # 4 groups per core

LAST_RESULTS = None


def _split_bir_waits(bir_json):
    """Reduce every instruction to at most one sync-wait command.

    The walrus build in this container rejects instructions with more than
    one sync wait. Two rewrites, semantics preserving:
    1. Drop waits on the instruction's own engine semaphore when another
       wait remains — engines retire instructions in order, so a wait on
       an earlier same-engine instruction is implied by program order.
    2. Split any remaining extra waits onto preceding same-engine Drain
       carriers (AND of conditions == sequential waits).
    """
    import json
    d = json.loads(bir_json)
    engs = ("PE", "DVE", "Activation", "Pool", "SP")
    for f in d["functions"]:
        for bb in f["blocks"]:
            out = []
            for ins in bb["instructions"]:
                si = ins.get("sync_info")
                if si:
                    waits = si.get("on_wait") or []
                    if len(waits) > 1:
                        eng = ins["engine"]
                        own = [w for w in waits
                               if w.get("ant_name", "").startswith(eng + "_")
                               and eng in engs]
                        foreign = [w for w in waits if w not in own]
                        if foreign:
                            waits = foreign
                        else:
                            waits = [waits[-1]]
                    if len(waits) > 1:
                        for i, w in enumerate(waits[:-1]):
                            out.append({
                                "debug": ins.get("debug", 0),
                                "engine": ins["engine"],
                                "ins": [], "outs": [],
                                "name": f"{ins['name']}_w{i}",
                                "opcode": "Drain",
                                "sync_info": {"on_update": [], "on_wait": [w]},
                            })
                        waits = [waits[-1]]
                    si["on_wait"] = waits
                out.append(ins)
            bb["instructions"] = out
    return json.dumps(d).encode()


_COMPILE_PATCHED = False


def _install_compile_patch():
    global _COMPILE_PATCHED
    if _COMPILE_PATCHED:
        return
    import concourse.bass_utils as _bu
    import concourse.bass2jax as _b2j
    _orig = _bu.compile_bir_kernel

    def _patched(bir_json, tmpdir, neff_name="file.neff", **kw):
        return _orig(_split_bir_waits(bir_json), tmpdir, neff_name=neff_name, **kw)

    _bu.compile_bir_kernel = _patched
    _b2j.compile_bir_kernel = _patched
    _COMPILE_PATCHED = True


def _kernel_numpy(u_vecs, W):
    u = u_vecs.astype(np.float32)
    w = W[0].astype(np.float32)
    uh = np.einsum('bsi,io->bso', u, w)
    uh = uh.reshape(B, S, NC_CAP, DC).transpose(0, 2, 1, 3)
    b = np.zeros((B, NC_CAP, S), dtype=np.float32)
    out = None
    for i in range(ROUT):
        m = b.max(axis=1, keepdims=True)
        e = np.exp(b - m)
        c = e / e.sum(axis=1, keepdims=True)
        o = np.einsum('bni,bnid->bnd', c, uh)
        out = o / np.sqrt((o * o).sum(-1, keepdims=True) + EPS)
        if i < ROUT - 1:
            b = np.einsum('bnd,bnid->bni', out, uh)
    return out.astype(np.float32)


def _const_blocks():
    """Host-built constant blocks DMA'd in as extra inputs."""
    import ml_dtypes
    bf16 = ml_dtypes.bfloat16
    # CB (bf16) [128, 289]: ident(128) | c0(128) | psel(32) | ones_col(1)
    cb = np.zeros((128, 289), dtype=np.float32)
    cb[:, 0:128] = np.eye(128, dtype=np.float32)
    cb[:, 128:256] = 1.0 / 16.0
    r = np.arange(128)
    cb[:, 256:288] = (r[:, None] % 32 == np.arange(32)[None, :]).astype(np.float32)
    cb[:, 288] = 1.0
    # CF (f32) [128, 640]: masks 4x128 | ones block 128
    cf = np.zeros((128, 640), dtype=np.float32)
    n_of_col = np.arange(128) % 16
    for ot in range(4):
        m = ((4 * ot + r[:, None] // 32) == n_of_col[None, :]).astype(np.float32)
        cf[:, 128 * ot:128 * (ot + 1)] = m
    cf[:, 512:640] = 1.0
    return cb.astype(bf16), cf


def _build_bass(bpc=BPC, repeat=1):
    import concourse.bass as bass
    import concourse.tile as tile
    from concourse import mybir
    from contextlib import ExitStack

    f32, bf16 = mybir.dt.float32, mybir.dt.bfloat16
    AF = mybir.ActivationFunctionType
    ng = bpc // NB

    nc = bass.Bass()
    u_d = nc.declare_dram_parameter("u", [bpc, S, DIN], f32, isOutput=False)
    w_d = nc.declare_dram_parameter("W", [1, DIN, O], f32, isOutput=False)
    cb_d = nc.declare_dram_parameter("CB", [128, 289], bf16, isOutput=False)
    cf_d = nc.declare_dram_parameter("CF", [128, 640], f32, isOutput=False)
    out_d = nc.declare_dram_parameter("out", [bpc, NC_CAP, DC], f32, isOutput=True)

    with ExitStack() as ctx:
        tc = ctx.enter_context(tile.TileContext(nc))
        const = ctx.enter_context(tc.tile_pool(name="const", bufs=1))
        sb_u = ctx.enter_context(tc.tile_pool(name="sb_u", bufs=1))
        sb_t = ctx.enter_context(tc.tile_pool(name="sb_t", bufs=1))
        sb_c = ctx.enter_context(tc.tile_pool(name="sb_c", bufs=1))
        work = ctx.enter_context(tc.tile_pool(name="work", bufs=2))
        ps_tp = ctx.enter_context(tc.tile_pool(name="ps_tp", bufs=1, space="PSUM"))
        ps_z = ctx.enter_context(tc.tile_pool(name="ps_z", bufs=3, space="PSUM"))
        ps_q = ctx.enter_context(tc.tile_pool(name="ps_q", bufs=4, space="PSUM"))

        # ---- constants ----
        cb = const.tile([128, 289], bf16, tag="cb")
        nc.sync.dma_start(cb[:], cb_d[:])
        cf = const.tile([128, 640], f32, tag="cf")
        nc.sync.dma_start(cf[:], cf_d[:])
        ident = cb[:, 0:128]
        c0 = cb[:, 128:256]
        psel = cb[:, 256:288]
        ones_col = cb[:, 288:289]
        mask3 = cf[:, 0:512].rearrange("p (ot c) -> p ot c", ot=4)
        onesf_row = cf[0:1, 512:640]
        epsc = const.tile([1, 1], f32, tag="epsc")
        nc.vector.memset(epsc[:], EPS)

        # ---- W load, cast, transpose ----
        wf = const.tile([128, 2, 512], f32, tag="wf")
        nc.sync.dma_start(wf[:], w_d[0].rearrange("(it p) o -> p it o", p=128))
        wbt = const.tile([128, 2, 512], bf16, tag="wbt")
        nc.vector.tensor_copy(wbt[:], wf[:])
        wtb = []
        for ot in range(4):
            ptw = ps_tp.tile([128, 2, 512], bf16, tag="tp")
            for it in range(2):
                nc.tensor.transpose(
                    ptw[:, 0, 128 * it:128 * (it + 1)],
                    wbt[:, it, 128 * ot:128 * (ot + 1)], ident)
            t = const.tile([128, 256], bf16, tag=f"wtb{ot}")
            nc.scalar.copy(t[:], ptw[:, 0, 0:256])
            wtb.append(t)

        ub = [None] * bpc
        uts = [None] * bpc
        cT = {}   # g -> [128, 4(st), NB, 16] bf16 tile (c for next iter)

        def prologue(g):
            for l in range(NB):
                b = NB * g + l
                uf = work.tile([128, 4, 256], f32, tag="uf", bufs=3)
                nc.sync.dma_start(uf[:], u_d[b].rearrange("(st p) i -> p st i", p=128))
                t = sb_u.tile([128, 4, 256], bf16, tag=f"ub{b}")
                if b % 3 == 0:
                    nc.vector.tensor_copy(t[:], uf[:])
                else:
                    nc.scalar.copy(t[:], uf[:])
                ub[b] = t
            for l in range(NB):
                b = NB * g + l
                ptp = ps_tp.tile([128, 2, 512], bf16, tag="tp")
                for it in range(2):
                    for st in range(4):
                        nc.tensor.transpose(
                            ptp[:, it, 128 * st:128 * (st + 1)],
                            ub[b][:, st, 128 * it:128 * (it + 1)], ident)
                t = sb_t.tile([128, 2, 512], bf16, tag=f"ut{b}")
                nc.vector.tensor_copy(t[:], ptp[:])
                uts[b] = t

        def crhs(g, k, st, l):
            if k == 0:
                return c0[:, 16 * l:16 * (l + 1)]
            return cT[g][:, st, l, :]

        def routing(g, k):
            # one PSUM bank per group-iter for ZT/norm/bcast/G (regions reused
            # sequentially; Tile subtile deps order the writers/readers)
            zfat = ps_z.tile([128, 3, 128], f32, tag="z")
            pz = zfat[:, 0:2, :]
            # ---- ZT[i,(l,n)] = sum_s u[s,i]*c[s,(l,n)] ----
            for it in range(2):
                for l in range(NB):
                    for st in range(4):
                        nc.tensor.matmul(
                            pz[:, it, 16 * l:16 * (l + 1)],
                            ub[NB * g + l][:, st, 128 * it:128 * (it + 1)],
                            crhs(g, k, st, l),
                            start=(st == 0), stop=(st == 3))
            zb = work.tile([128, 2, 128], bf16, tag="zb", bufs=3)
            nc.vector.tensor_copy(zb[:], pz[:])
            # ---- P[o,(l,n)] = sum_i W[i,o]*ZT[i,(l,n)];  V = P*mask ----
            pp = ps_q.tile([128, 4, 128], f32, tag="q")
            for ot in range(4):
                for it in range(2):
                    nc.tensor.matmul(pp[:, ot, :],
                                     wbt[:, it, 128 * ot:128 * (ot + 1)],
                                     zb[:, it, :], start=(it == 0), stop=(it == 1))
            vb = work.tile([128, 4, 128], bf16, tag="vb", bufs=3)
            nc.vector.tensor_mul(vb[:], pp[:], mask3)
            vsq = work.tile([128, 4, 128], bf16, tag="vsq")
            nc.scalar.activation(vsq[:], vb[:], AF.Square)
            # ---- |V|^2 col sums (accumulate the 4 o-tiles into [1,128]) ----
            pn = zfat[0:1, 2, :]
            for ot in range(4):
                nc.tensor.matmul(pn, ones_col, vsq[:, ot, :],
                                 start=(ot == 0), stop=(ot == 3))
            sq = work.tile([1, 128], f32, tag="sq")
            nc.scalar.activation(sq[:], pn, AF.Sqrt, bias=epsc[:])
            rsn = work.tile([1, 128], f32, tag="rsn")
            nc.vector.reciprocal(rsn[:], sq[:])
            pbc = zfat[:, 2, :]
            nc.tensor.matmul(pbc, onesf_row, rsn[:], start=True, stop=True)
            snsb = work.tile([128, 128], f32, tag="snsb")
            nc.scalar.copy(snsb[:], pbc)
            if k < ROUT - 1:
                # ---- G[i,(l,n)] = sum_o W[i,o]*V[o,(l,n)], scaled by rsqrt ----
                pg = zfat[:, 0:2, :]
                for it in range(2):
                    for ot in range(4):
                        nc.tensor.matmul(pg[:, it, :],
                                         wtb[ot][:, 128 * it:128 * (it + 1)],
                                         vb[:, ot, :], start=(ot == 0), stop=(ot == 3))
                gb = work.tile([128, 2, 128], bf16, tag="gb")
                nc.vector.tensor_mul(
                    gb[:], pg[:], snsb[:].unsqueeze(1).broadcast_to((128, 2, 128)))
                # ---- bT[s,(l,n)] = sum_i u[s,i]*G[i,(l,n)] (= b*rsqrt) ----
                pbt = ps_q.tile([128, 4, 128], f32, tag="q")
                for st in range(4):
                    for l in range(NB):
                        for it in range(2):
                            nc.tensor.matmul(
                                pbt[:, st, 16 * l:16 * (l + 1)],
                                uts[NB * g + l][:, it, 128 * st:128 * (st + 1)],
                                gb[:, it, 16 * l:16 * (l + 1)],
                                start=(it == 0), stop=(it == 1))
                # softmax over n (16-blocks), scale-free (c ~ 16*softmax)
                e = work.tile([128, 4, NB, 16], f32, tag="e", bufs=3)
                nc.scalar.activation(
                    e[:], pbt[:].rearrange("p st (l n) -> p st l n", n=16), AF.Exp)
                d8 = work.tile([128, 4, NB, 8], f32, tag="d8")
                nc.vector.tensor_add(d8[:], e[:, :, :, 0:8], e[:, :, :, 8:16])
                d4 = work.tile([128, 4, NB, 4], f32, tag="d4")
                nc.vector.tensor_add(d4[:], d8[:, :, :, 0:4], d8[:, :, :, 4:8])
                d2 = work.tile([128, 4, NB, 2], f32, tag="d2")
                nc.vector.tensor_add(d2[:], d4[:, :, :, 0:2], d4[:, :, :, 2:4])
                d1 = work.tile([128, 4, NB, 1], f32, tag="d1")
                nc.vector.tensor_add(d1[:], d2[:, :, :, 0:1], d2[:, :, :, 1:2])
                rr = work.tile([128, 4, NB, 1], f32, tag="rr")
                nc.vector.reciprocal(rr[:], d1[:])
                ct = sb_c.tile([128, 4, NB, 16], bf16, tag=f"c{g}_{k % 2}")
                nc.vector.tensor_mul(
                    ct[:], e[:], rr[:].broadcast_to((128, 4, NB, 16)))
                cT[g] = ct
            else:
                # ---- final: vs = V*rsqrt; extract [(l,n), d] via psel ----
                vs = work.tile([128, 4, 128], bf16, tag="vs")
                nc.vector.tensor_mul(
                    vs[:], vb[:], snsb[:].unsqueeze(1).broadcast_to((128, 4, 128)))
                pout = ps_q.tile([128, 4, 128], f32, tag="q")
                for ot in range(4):
                    nc.tensor.matmul(pout[:, 0, 0:32], vs[:, ot, :], psel,
                                     start=(ot == 0), stop=(ot == 3))
                osb = work.tile([128, 32], f32, tag="osb")
                nc.scalar.copy(osb[:], pout[:, 0, 0:32])
                nc.sync.dma_start(
                    out_d[NB * g:NB * (g + 1)].rearrange("b n d -> (b n) d"),
                    osb[:])

        # software-pipelined emission: prologue(g) at step g, iter k at step g+1+k
        for _rep in range(repeat):
            for step in range(ng + ROUT):
                for g in range(ng):
                    p = step - g
                    if p == 0:
                        prologue(g)
                    elif 1 <= p <= ROUT:
                        routing(g, p - 1)
    return nc


def kernel(u_vecs, W):
    global LAST_RESULTS
    try:
        _install_compile_patch()
        from concourse.bass_utils import run_bass_kernel_spmd
        nc = _build_bass()
        cb, cf = _const_blocks()
        u = np.ascontiguousarray(u_vecs, dtype=np.float32)
        w = np.ascontiguousarray(W, dtype=np.float32)
        in_maps = [
            {"u": u[c * BPC:(c + 1) * BPC], "W": w, "CB": cb, "CF": cf}
            for c in range(NCORES)
        ]
        res = run_bass_kernel_spmd(nc, in_maps, core_ids=list(range(NCORES)))
        LAST_RESULTS = res
        out = np.concatenate([res.results[c]["out"] for c in range(NCORES)], axis=0)
        return out.astype(np.float32)
    except Exception as ex:
        import traceback
        traceback.print_exc(file=sys.stderr)
        sys.stderr.write(f"[kernel.py] bass path failed ({ex!r}); numpy fallback\n")
        return _kernel_numpy(u_vecs, W)
